# revision 5
# baseline (speedup 1.0000x reference)
"""Trainium2 Bass kernel for nn_CrossAttentionLayer (cross attention with
materialized attention-probability output).

Sharding: core = (batch b, head-pair hp).  8 cores = 2 batches x 4 head pairs.
Each core computes, for its batch and its 2 heads:
  Qh = Wq_h q ; Kh = Wk_h k ; Vt = (Wv_h v)^T  (ones column appended)
  S^T[q,p] = Kh^T Qh            (energy, transposed layout: q on partitions)
  P'' = exp(S^T/16 - ln 4096)   (ScalarE, range-shifted softmax numerator)
  AV[d,p], Z[p] accumulated on PE via the ones column of Vt
  A^T = P'' * (1/Z)             (in-place DVE multiply, fp16, DMA'd out)
  out_partial = Wo_slice @ (AV/Z)  (per-core partial of the output projection)
Host: sums out partials per batch (+bo), transposes/upcasts A blocks.
"""

import math
import sys

sys.path.insert(0, "/opt/trn_rl_repo")

import numpy as np

B, C, NH, HD = 2, 256, 8, 32
H = W = 32
T = 8
P = H * W              # 1024 query positions
Q = T * H * W          # 8192 key positions
NQT = Q // 128         # 64 q-tiles of 128
PB = 512               # p-block (2 per core)
SCALE = 1.0 / 16.0     # 1/sqrt(NH*HD)
LN_SHIFT = math.log(4096.0)
NCORES = 8

_CACHE = {}


def _build():
    import concourse.tile as tile
    from concourse import bacc, mybir

    f16 = mybir.dt.float16
    f32 = mybir.dt.float32
    AF = mybir.ActivationFunctionType

    nc = bacc.Bacc("TRN2", target_bir_lowering=False, debug=False,
                   num_devices=NCORES)

    # ---- DRAM I/O (per-core shapes; host pre-arranges layouts) ----
    q_in = nc.dram_tensor("q_in", [128, 2, P], f16, kind="ExternalInput")
    k_in = nc.dram_tensor("k_in", [128, 2, Q], f16, kind="ExternalInput")
    v_in = nc.dram_tensor("v_in", [128, 2, Q], f16, kind="ExternalInput")
    wq_in = nc.dram_tensor("wq_in", [128, 2, 64], f16, kind="ExternalInput")
    wk_in = nc.dram_tensor("wk_in", [128, 2, 64], f16, kind="ExternalInput")
    wv_in = nc.dram_tensor("wv_in", [128, 2, 64], f16, kind="ExternalInput")
    wo_in = nc.dram_tensor("wo_in", [32, 2, 256], f32, kind="ExternalInput")
    bq_in = nc.dram_tensor("bq_in", [64, 1], f32, kind="ExternalInput")
    bk_in = nc.dram_tensor("bk_in", [64, 1], f32, kind="ExternalInput")
    bv_in = nc.dram_tensor("bv_in", [128, 64], f32, kind="ExternalInput")
    a_out = nc.dram_tensor("a_out", [2, Q, P], f16, kind="ExternalOutput")
    o_out = nc.dram_tensor("o_out", [256, P], f32, kind="ExternalOutput")

    with tile.TileContext(nc) as tc:
        with tc.tile_pool(name="persist", bufs=1) as persist:

            # persistent SBUF tensors
            kh_sb = persist.tile([64, Q], f16, tag="kh")       # [32hh+d, q]
            qh_sb = persist.tile([64, P], f16, tag="qh")       # [32hh+d, p]
            vt_sb = persist.tile([128, NQT, 66], f16, tag="vt")  # [q0, k, V0|1|V1]
            outh = persist.tile([33, 2, P], f32, tag="outh")
            wo_sb = persist.tile([32, 2, 256], f32, tag="wo")
            bq_sb = persist.tile([64, 1], f32, tag="bq")
            bk_sb = persist.tile([64, 1], f32, tag="bk")
            bv_sb = persist.tile([128, 64], f32, tag="bv")
            ebias = persist.tile([128, 1], f32, tag="ebias")
            ones_t = persist.tile([33, 128], f32, tag="ones")

            nc.vector.memset(ebias[:], -LN_SHIFT)
            nc.vector.memset(ones_t[:], 1.0)
            nc.vector.memset(vt_sb[:, :, 32:33], 1.0)  # ones column head 0
            nc.vector.memset(vt_sb[:, :, 65:66], 1.0)  # ones column head 1
            nc.sync.dma_start(wo_sb[:], wo_in.ap())
            nc.sync.dma_start(bq_sb[:], bq_in.ap())
            nc.sync.dma_start(bk_sb[:], bk_in.ap())
            nc.sync.dma_start(bv_sb[:], bv_in.ap())

            # ---------------- projections ----------------
            with tc.tile_pool(name="kv", bufs=1) as kvp, \
                 tc.tile_pool(name="projps", bufs=1, space="PSUM") as projps, \
                 tc.tile_pool(name="projps2", bufs=2, space="PSUM") as projps2:
                wq_sb = kvp.tile([128, 2, 64], f16, tag="wq")
                wk_sb = kvp.tile([128, 2, 64], f16, tag="wk")
                wv_sb = kvp.tile([128, 2, 64], f16, tag="wv")
                q_sb = kvp.tile([128, 2, P], f16, tag="qsb")
                k_sb = kvp.tile([128, 2, Q], f16, tag="ksb")
                v_sb = kvp.tile([128, 2, Q], f16, tag="vsb")
                nc.sync.dma_start(wq_sb[:], wq_in.ap())
                nc.sync.dma_start(wk_sb[:], wk_in.ap())
                nc.sync.dma_start(wv_sb[:], wv_in.ap())
                nc.sync.dma_start(q_sb[:], q_in.ap())
                nc.sync.dma_start(k_sb[:], k_in.ap())
                nc.sync.dma_start(v_sb[:], v_in.ap())

                # Qh: psum [64, P]
                psq = projps.tile([64, P], f32, tag="psq")
                for pc in range(P // 512):
                    for half in range(2):
                        nc.tensor.matmul(
                            psq[:, pc * 512:(pc + 1) * 512],
                            wq_sb[:, half, :],
                            q_sb[:, half, pc * 512:(pc + 1) * 512],
                            start=(half == 0), stop=(half == 1))
                nc.scalar.activation(qh_sb[:], psq[:], AF.Identity,
                                     bias=bq_sb[:], scale=1.0)

                # Kh: 4 chunks of 2048
                for qc in range(4):
                    psk = projps.tile([64, 2048], f32, tag="psk")
                    for sc in range(4):
                        lo = qc * 2048 + sc * 512
                        for half in range(2):
                            nc.tensor.matmul(
                                psk[:, sc * 512:(sc + 1) * 512],
                                wk_sb[:, half, :],
                                k_sb[:, half, lo:lo + 512],
                                start=(half == 0), stop=(half == 1))
                    nc.scalar.activation(
                        kh_sb[:, qc * 2048:(qc + 1) * 2048], psk[:],
                        AF.Identity, bias=bk_sb[:], scale=1.0)

                # Vt: per q-tile [128, 64] = v_tile^T @ Wv^T
                for k in range(NQT):
                    psv = projps2.tile([128, 64], f32, tag="psv")
                    for half in range(2):
                        nc.tensor.matmul(
                            psv[:],
                            v_sb[:, half, k * 128:(k + 1) * 128],
                            wv_sb[:, half, :],
                            start=(half == 0), stop=(half == 1))
                    nc.vector.tensor_add(vt_sb[:, k, 0:32], psv[:, 0:32],
                                         bv_sb[:, 0:32])
                    nc.vector.tensor_add(vt_sb[:, k, 33:65], psv[:, 32:64],
                                         bv_sb[:, 32:64])

            # ---------------- attention units ----------------
            SLOTS = [list(range(3 * s, min(3 * s + 3, NQT)))
                     for s in range((NQT + 2) // 3)]

            ctx2 = tc.tile_pool(name="pp", bufs=2)
            pp_pool = ctx2.__enter__()
            ctx3 = tc.tile_pool(name="unitbuf", bufs=2)
            unitbuf = ctx3.__enter__()
            ctx4 = tc.tile_pool(name="avps", bufs=1, space="PSUM")
            avps = ctx4.__enter__()
            ctx5 = tc.tile_pool(name="eps", bufs=2, space="PSUM")
            eps = ctx5.__enter__()
            for hh in range(2):
                for pbi in range(2):
                    pb = pbi * PB
                    pps = pp_pool.tile([128, NQT, PB], f16, tag="ppbuf")
                    av = avps.tile([33, PB], f32, tag="av")
                    for slot in SLOTS:
                        pse = eps.tile([128, 1536], f32, tag="pse")
                        for j, k in enumerate(slot):
                            nc.tensor.matmul(
                                pse[:, j * 512:(j + 1) * 512],
                                kh_sb[32 * hh:32 * hh + 32,
                                      k * 128:(k + 1) * 128],
                                qh_sb[32 * hh:32 * hh + 32, pb:pb + PB],
                                start=True, stop=True)
                        n = len(slot) * 512
                        nc.scalar.activation(
                            pps[:, slot[0]:slot[0] + len(slot), :],
                            pse[:, 0:n].rearrange("p (a b) -> p a b", b=PB),
                            AF.Exp, bias=ebias[:], scale=SCALE)
                        for k in slot:
                            nc.tensor.matmul(
                                av[0:33, :],
                                vt_sb[:, k, 33 * hh:33 * hh + 33],
                                pps[:, k, :],
                                start=(k == 0), stop=(k == NQT - 1),
                                skip_group_check=True)

                    # phase 2: normalize + outputs
                    rz = unitbuf.tile([33, PB], f32, tag="rz")
                    rb16 = unitbuf.tile([128, PB], f16, tag="rb16")
                    rb32 = unitbuf.tile([128, PB], f32, tag="rb32")
                    psr = avps.tile([128, 512], f32, tag="psr")
                    nc.vector.reciprocal(rz[32:33, :], av[32:33, :])
                    nc.tensor.matmul(psr[:, 0:PB], ones_t[32:33, :],
                                     rz[32:33, :], start=True, stop=True)
                    nc.vector.tensor_copy(rb16[:], psr[:, 0:PB])
                    nc.vector.tensor_copy(rb32[:], psr[:, 0:PB])
                    nc.vector.tensor_mul(outh[0:32, hh, pb:pb + PB],
                                         av[0:32, :], rb32[0:32, :])
                    rbb = rb16[:].unsqueeze(1).broadcast_to((128, NQT, PB))
                    nc.vector.tensor_mul(pps[:], pps[:], rbb)
                    nc.sync.dma_start(
                        a_out.ap()[hh].rearrange(
                            "(k p) c -> p k c", p=128)[:, :, pb:pb + PB],
                        pps[:])

            ctx5.__exit__(None, None, None)
            ctx4.__exit__(None, None, None)
            ctx3.__exit__(None, None, None)
            ctx2.__exit__(None, None, None)

            # ---------------- output projection ----------------
            with tc.tile_pool(name="ops", bufs=2, space="PSUM") as ops, \
                 tc.tile_pool(name="osb", bufs=1) as osb:
                osl = osb.tile([128, 2, P], f32, tag="osl")
                for m in range(2):
                    pso = ops.tile([128, P], f32, tag="pso")
                    for pc in range(2):
                        for hh in range(2):
                            nc.tensor.matmul(
                                pso[:, pc * 512:(pc + 1) * 512],
                                wo_sb[:, hh, m * 128:(m + 1) * 128],
                                outh[0:32, hh, pc * 512:(pc + 1) * 512],
                                start=(hh == 0), stop=(hh == 1))
                    nc.vector.tensor_copy(osl[:, m, :], pso[:])
                nc.sync.dma_start(
                    o_out.ap().rearrange("(m p) c -> p m c", p=128), osl[:])

    nc.compile()
    return nc


def _get_nc():
    if "nc" not in _CACHE:
        _CACHE["nc"] = _build()
    return _CACHE["nc"]


def _prep_core_inputs(query, key, value, Wq, bq, Wk, bk, Wv, bv, Wo, bo):
    """Host-side slicing/layout per core. core = 4*b + hp."""
    maps = []
    for core in range(NCORES):
        b, hp = divmod(core, 4)
        cs = slice(64 * hp, 64 * hp + 64)

        def chalf(x2d):  # [256, N] f32 -> [128, 2, N] f16
            return np.ascontiguousarray(
                x2d.reshape(2, 128, -1).transpose(1, 0, 2)).astype(np.float16)

        wo_sl = np.zeros((32, 2, 256), np.float32)
        wo_sl[:, 0, :] = Wo[:, 64 * hp:64 * hp + 32].T
        wo_sl[:, 1, :] = Wo[:, 64 * hp + 32:64 * hp + 64].T
        maps.append({
            "q_in": chalf(query[b].reshape(C, P)),
            "k_in": chalf(key[b].reshape(C, Q)),
            "v_in": chalf(value[b].reshape(C, Q)),
            "wq_in": chalf(Wq[cs].T),
            "wk_in": chalf(Wk[cs].T),
            "wv_in": chalf(Wv[cs].T),
            "wo_in": wo_sl,
            "bq_in": bq[cs].astype(np.float32).reshape(64, 1),
            "bk_in": bk[cs].astype(np.float32).reshape(64, 1),
            "bv_in": np.tile(bv[cs].astype(np.float32), (128, 1)),
        })
    return maps


def kernel(**inputs):
    from concourse import bass_utils

    inputs = {k: np.asarray(v) for k, v in inputs.items()}
    nc = _get_nc()
    in_maps = _prep_core_inputs(**inputs)
    res = bass_utils.run_bass_kernel_spmd(nc, in_maps,
                                          core_ids=list(range(NCORES)))

    bo = inputs["bo"].astype(np.float32)
    out = np.zeros((B, 256, P), np.float32)
    A = np.empty((B, P, Q, NH), np.float32)
    for core in range(NCORES):
        b, hp = divmod(core, 4)
        out[b] += res.results[core]["o_out"]
        a_dev = res.results[core]["a_out"]  # [2, Q, P] f16
        for hhh in range(2):
            A[b, :, :, 2 * hp + hhh] = a_dev[hhh].T.astype(np.float32)
    out += bo[None, :, None]
    return out.reshape(B, 256, H, W), A
